# revision 36
# baseline (speedup 1.0000x reference)
"""Trainium2 Bass kernel for nn_BrainAttention_69707319214147.

Model (reference.py): masked-weight QKV projections, per-row top-256-of-1024
sparsified attention scores, softmax over the scatter-into-zeros matrix
(zeros contribute exp(0)=1), AV, masked-weight output projection.

Sharding: 8 cores = 4 batches x 2 head-groups. Core i handles batch i//2 and
heads (i%2)*8 .. +8. Each core computes a partial output projection over its
512 y-channels; the host sums partner-core partials and adds bias terms.

Weight prep (w*mask fold, transposes, fp16 casts) happens on the host; the
device runs an all-fp16 pipeline (fp32 PSUM accumulation everywhere).

Per-head top-k softmax: tau approximates the 256th-largest raw score via a
Gaussian-quantile init (mean from the ACT copy accumulator, fixed global
sigma) plus 5 damped exact-count secant rounds - each a single fused fp16
compare+count tensor_scalar per row-block, 4x DVE mode. Then
u = (S >= tau)*S via a compare mask (DVE) and mask*S multiply (gpsimd),
dd = Exp(u/8) on ACT with the exact softmax denominator Z as the same
instruction's free accumulator (rejected entries give exp(0)=1 exactly),
dd scaled by 1/Z on DVE, transposed through the DMA xbar in one batched
[128,1024] call per row-block, and fed to AV / o-proj matmuls in fp16.
"""
import numpy as np
from collections import deque
from contextlib import ExitStack

import concourse.bass as bass
import concourse.mybir as mybir
import concourse.tile as tile
from concourse import bacc, bass_utils

F32 = mybir.dt.float32
F16 = mybir.dt.float16
AF = mybir.ActivationFunctionType
ALU = mybir.AluOpType

B, T, C, H = 4, 1024, 1024, 16
D = C // H            # 64
NCORE = 8
HPC = H // 2          # heads per core = 8
NT = T // 128         # 8 t-tiles
NCH = C // 128        # 8 contraction chunks
Z0 = 0.6744897501960817          # Phi^-1(0.75)
PHI0 = 0.3177765798295446        # phi(Z0)
SIGMA_GLOB = 4.0                 # global score-sigma estimate
SLOPE = SIGMA_GLOB / (T * PHI0)  # d(tau)/d(count)
TAU0_OFF = Z0 * SIGMA_GLOB
DAMPS = (1.0, 0.75, 0.5)  # damped secant update rounds


def _build_body(ctx, tc, io):
    nc = tc.nc
    P = 128

    # ---------------- persistent tiles ----------------
    pers = ctx.enter_context(tc.tile_pool(name="pers", bufs=1))

    # host-prepped fp16 effective weights + x
    weffq = pers.tile([P, NCH, 512], F16, tag="weffq")
    nc.sync.dma_start(weffq, io["wqt"].rearrange("(a p) f -> p a f", p=P))
    weffk = pers.tile([P, NCH, 512], F16, tag="weffk")
    nc.sync.dma_start(weffk, io["wkt"].rearrange("(a p) f -> p a f", p=P))
    xT16 = pers.tile([P, NCH, T], F16, tag="xT16")
    nc.sync.dma_start(xT16, io["xT"].rearrange("(a p) f -> p a f", p=P))

    bqc = pers.tile([P, 4], F32, tag="bqc")
    nc.sync.dma_start(bqc, io["bqs"].rearrange("(a p) -> p a", p=P))
    bkc = pers.tile([P, 4], F32, tag="bkc")
    nc.sync.dma_start(bkc, io["bks"].rearrange("(a p) -> p a", p=P))
    weffv = pers.tile([P, NCH, 512], F16, tag="weffv")
    nc.sync.dma_start(weffv, io["wvt"].rearrange("(a p) f -> p a f", p=P))
    weffo = pers.tile([P, 4, T], F16, tag="weffo")
    nc.sync.dma_start(weffo, io["wot"].rearrange("(a p) f -> p a f", p=P))

    qT = [pers.tile([P, T], F16, tag=f"qT{p}", name=f"qT{p}") for p in range(4)]
    kT = [pers.tile([P, T], F16, tag=f"kT{p}", name=f"kT{p}") for p in range(4)]
    vbf = [pers.tile([P, 512], F16, tag=f"v{ti}", name=f"v{ti}")
           for ti in range(NT)]
    yTp = [pers.tile([P, T], F16, tag=f"yTp{p}", name=f"yTp{p}")
           for p in range(4)]

    # ---------------- main loop: projections + attention ----------------
    with ExitStack() as c3:
        pps = c3.enter_context(tc.tile_pool(name="pps", bufs=2, space="PSUM"))
        pp2 = c3.enter_context(tc.tile_pool(name="pp2", bufs=1, space="PSUM"))
        avp = c3.enter_context(tc.tile_pool(name="avp", bufs=1, space="PSUM"))
        Spool = c3.enter_context(tc.tile_pool(name="Spool", bufs=17))
        cop = c3.enter_context(tc.tile_pool(name="cop", bufs=3))
        mpool = c3.enter_context(tc.tile_pool(name="mpool", bufs=2))
        upool = c3.enter_context(tc.tile_pool(name="upool", bufs=3))
        dpool = c3.enter_context(tc.tile_pool(name="dpool", bufs=4))
        dts = c3.enter_context(tc.tile_pool(name="dts", bufs=3))
        smp = c3.enter_context(tc.tile_pool(name="smp", bufs=4))
        zpool = c3.enter_context(tc.tile_pool(name="zpool", bufs=8))

        # PE backlog: proj/v/AV matmuls queue here and drain interleaved
        # between score tiles so long bursts never delay the score copies
        # that feed the ACT/DVE pipeline.
        backlog = deque()
        nfill = [0]
        ndrain = [0]

        def drain(n):
            while backlog and n > 0:
                backlog.popleft()()
                ndrain[0] += 1
                n -= 1

        def drain_to(mark):
            while ndrain[0] < mark and backlog:
                backlog.popleft()()
                ndrain[0] += 1

        def fill(thunk):
            backlog.append(thunk)
            nfill[0] += 1

        def fill_proj_quarter(p):
            for nm, wt, dst, bias in (("q", weffq, qT, bqc), ("k", weffk, kT, bkc)):
                ps = pp2.tile([P, T], F32, tag="ps2", name=f"ps2{nm}{p}")
                for cj in range(NCH):
                    for nh in range(2):
                        fill(lambda ps=ps, cj=cj, nh=nh, wt=wt: nc.tensor.matmul(
                            ps[:, nh * 512:(nh + 1) * 512],
                            lhsT=wt[:, cj, p * P:(p + 1) * P],
                            rhs=xT16[:, cj, nh * 512:(nh + 1) * 512],
                            start=(cj == 0), stop=(cj == NCH - 1),
                        ))
                fill(lambda ps=ps, dst=dst, bias=bias: nc.scalar.activation(
                    dst[p], ps, AF.Identity, bias=bias[:, p:p + 1], scale=1.0))
            return nfill[0]

        av_tiles = []

        def fill_av_half(yps, g, half):
            hh = 2 * g + half
            dts_h = av_tiles[hh]
            for nh in range(2):
                for j in range(NT):
                    fill(lambda yps=yps, half=half, hh=hh, nh=nh, j=j,
                         dts_h=dts_h: nc.tensor.matmul(
                        yps[64 * half:64 * half + 64,
                            nh * 512:(nh + 1) * 512],
                        lhsT=vbf[j][:, 64 * hh:64 * hh + 64],
                        rhs=dts_h[:, j, nh * 512:(nh + 1) * 512],
                        start=(j == 0), stop=(j == NT - 1),
                    ))

        def fill_av_pair(g):
            yps = avp.tile([P, T], F32, tag="yps", name=f"yps{g}")
            fill_av_half(yps, g, 0)
            fill_av_half(yps, g, 1)
            fill(lambda yps=yps, g=g: nc.scalar.copy(yTp[g], yps))

        def fill_v_pair(ti0):
            ps = pp2.tile([P, T], F32, tag="ps2", name=f"ps2v{ti0}")
            for half in range(2):
                ti = ti0 + half
                for cj in range(NCH):
                    fill(lambda ps=ps, half=half, ti=ti, cj=cj: nc.tensor.matmul(
                        ps[:, half * 512:(half + 1) * 512],
                        lhsT=xT16[:, cj, ti * P:(ti + 1) * P],
                        rhs=weffv[:, cj, :],
                        start=(cj == 0), stop=(cj == NCH - 1),
                    ))
                fill(lambda ps=ps, half=half, ti=ti: nc.scalar.copy(
                    vbf[ti], ps[:, half * 512:(half + 1) * 512]))

        def emit_scores(h):
            # copies for ti in dve_set are deferred to emit_dve_copies (run
            # on DVE after the previous head's counts) to balance ACT/DVE
            p, off = h // 2, 64 * (h % 2)
            dve_set = () if h == 0 else (6, 7)
            Ssb = []
            pend = []
            for ti in range(NT):
                ps = pps.tile([P, T], F32, tag="ps")
                for nh in range(2):
                    nc.tensor.matmul(
                        ps[:, nh * 512:(nh + 1) * 512],
                        lhsT=qT[p][off:off + 64, ti * P:(ti + 1) * P],
                        rhs=kT[p][off:off + 64, nh * 512:(nh + 1) * 512],
                        start=True, stop=True,
                    )
                ssb = Spool.tile([P, T], F16, tag="ssb")
                if ti in dve_set:
                    pend.append((ssb, ps))
                else:
                    nc.scalar.copy(ssb, ps)
                Ssb.append(ssb)
                drain(7)
            return Ssb, pend

        def emit_dve_copies(pend):
            for ssb, ps in pend:
                nc.vector.tensor_copy(ssb, ps)

        # PE warm-up: dummy matmuls bridge the initial weight-DMA wait so the
        # tensor engine reaches full pstate before the first projection
        warm = Spool.tile([P, 512], F16, tag="warm", name="warm")
        nc.gpsimd.memset(warm, 0.0)
        wps = pp2.tile([P, 512], F32, tag="ps2", name="warmps")
        for _ in range(56):
            nc.tensor.matmul(wps, lhsT=warm[:, 0:128], rhs=warm,
                             start=True, stop=True)

        # prologue: head 0's production (quarter 0 runs eagerly; the PE is
        # idle during the weight DMAs anyway)
        fill_proj_quarter(0)
        drain(10 ** 9)
        fill_v_pair(0)
        fill_v_pair(2)
        Scur, pend0 = emit_scores(0)
        emit_dve_copies(pend0)

        proj_marks = {}
        for h in range(HPC):
            # fill the PE backlog for upcoming blocks: quarter p two blocks
            # ahead of its first consumer, AV pair g one block after its
            # second head completes
            if h == 0:
                fill_v_pair(4)
                fill_v_pair(6)
                proj_marks[1] = fill_proj_quarter(1)
            elif h in (2, 4):
                proj_marks[h // 2 + 1] = fill_proj_quarter(h // 2 + 1)
            if h >= 2 and h % 2 == 0:
                fill_av_pair((h - 2) // 2)
            if h == HPC - 1:
                # last pair: head-6 half can run while head 7's tail drains
                av_last = avp.tile([P, T], F32, tag="yps", name="yps3")
                fill_av_half(av_last, HPC // 2 - 1, 0)

            pend_next = []
            if h + 1 < HPC:
                if (h + 1) % 2 == 0:
                    drain_to(proj_marks[(h + 1) // 2])
                Snext, pend_next = emit_scores(h + 1)
            Ssb = Scur

            # --- damped exact-count secant rounds (fixed global init) ---
            tau = None
            for r, damp in enumerate(DAMPS):
                cnt = smp.tile([P, NT], F32, tag="cnt")
                for ti in range(NT):
                    jk = cop.tile([P, T], F16, tag="jk")
                    nc.vector.tensor_scalar(
                        jk, Ssb[ti],
                        TAU0_OFF if r == 0 else tau[:, ti:ti + 1], None,
                        op0=ALU.is_ge, op1=ALU.add,
                        accum_out=cnt[:, ti:ti + 1])
                dl = smp.tile([P, NT], F32, tag="dl")
                nc.vector.tensor_scalar(dl, cnt, -256.0, damp * SLOPE,
                                        op0=ALU.add, op1=ALU.mult)
                tau2 = smp.tile([P, NT], F32, tag="tau")
                if r == 0:
                    nc.vector.tensor_scalar_add(tau2, dl, TAU0_OFF)
                else:
                    nc.vector.tensor_add(tau2, tau, dl)
                tau = tau2

            # --- select, exp (with exact Z), normalize, transpose ---
            DTs = dts.tile([P, NT, T], F16, tag="dts")
            zacc = smp.tile([P, NT], F32, tag="zacc")
            for ti in range(NT):
                mk = mpool.tile([P, T], F16, tag="mk")
                nc.vector.tensor_scalar(mk, Ssb[ti], tau[:, ti:ti + 1], None,
                                        op0=ALU.is_ge)
                u = upool.tile([P, T], F16, tag="u")
                nc.vector.tensor_tensor(out=u, in0=mk, in1=Ssb[ti],
                                        op=ALU.mult)
                dd = dpool.tile([P, T], F16, tag="dd")
                nc.scalar.activation(dd, u, AF.Exp, scale=0.125,
                                     accum_out=zacc[:, ti:ti + 1])
                zinv = zpool.tile([P, 1], F32, tag="zinv")
                nc.vector.reciprocal(zinv, zacc[:, ti:ti + 1])
                if ti % 2 == 0 or h == HPC - 1:
                    nc.gpsimd.tensor_scalar_mul(dd, dd, zinv[:, 0:1])
                else:
                    nc.vector.tensor_scalar_mul(dd, dd, zinv[:, 0:1])
                nc.sync.dma_start_transpose(
                    DTs[:, :, ti * P:(ti + 1) * P], dd)

            # deferred DVE copies for head h+1 (end of block: psum ready)
            emit_dve_copies(pend_next)

            av_tiles.append(DTs)
            if h == HPC - 1:
                fill_av_half(av_last, HPC // 2 - 1, 1)
                fill(lambda: nc.scalar.copy(yTp[HPC // 2 - 1], av_last))
                drain(10 ** 9)
            if h + 1 < HPC:
                Scur = Snext

    # ---------------- output projection ----------------
    with ExitStack() as c4:
        ops4 = c4.enter_context(tc.tile_pool(name="ops4", bufs=2, space="PSUM"))
        ost4 = c4.enter_context(tc.tile_pool(name="ost4", bufs=2))
        for ti in range(NT):
            ps = ops4.tile([P, T], F32, tag="ops")
            for cj in range(4):
                for nh in range(2):
                    nc.tensor.matmul(
                        ps[:, nh * 512:(nh + 1) * 512],
                        lhsT=yTp[cj][:, ti * P:(ti + 1) * P],
                        rhs=weffo[:, cj, nh * 512:(nh + 1) * 512],
                        start=(cj == 0), stop=(cj == 3),
                    )
            ost = ost4.tile([P, T], F16, tag="ost")
            nc.scalar.copy(ost, ps)
            nc.sync.dma_start(io["out_part"][ti * P:(ti + 1) * P, :], ost)


_PROG_CACHE = {}


def _build_program():
    if "nc" in _PROG_CACHE:
        return _PROG_CACHE["nc"]
    nc = bacc.Bacc("TRN2", target_bir_lowering=False, debug=False)
    io = {}
    io["xT"] = nc.dram_tensor("xT", [C, T], F16, kind="ExternalInput").ap()
    for nm in ("q", "k", "v"):
        io[f"w{nm}t"] = nc.dram_tensor(f"w{nm}t", [C, 512], F16,
                                       kind="ExternalInput").ap()
    io["wot"] = nc.dram_tensor("wot", [512, C], F16, kind="ExternalInput").ap()
    io["bqs"] = nc.dram_tensor("bqs", [512], F32, kind="ExternalInput").ap()
    io["bks"] = nc.dram_tensor("bks", [512], F32, kind="ExternalInput").ap()
    io["out_part"] = nc.dram_tensor("out_part", [T, C], F16,
                                    kind="ExternalOutput").ap()
    with tile.TileContext(nc) as tc:
        with ExitStack() as ctx:
            _build_body(ctx, tc, io)
    nc.compile()
    _PROG_CACHE["nc"] = nc
    return nc


def _in_maps(inputs):
    f32 = np.float32
    x = np.asarray(inputs["x"], f32)
    weff = {}
    for nm in ("q", "k", "v", "o"):
        weff[nm] = (np.asarray(inputs["w" + nm], f32)
                    * np.asarray(inputs["m" + nm], f32))
    bq, bk = np.asarray(inputs["bq"], f32), np.asarray(inputs["bk"], f32)
    maps = []
    for core in range(NCORE):
        b, g = core // 2, core % 2
        hs = g * 512
        maps.append({
            "xT": np.ascontiguousarray(x[b].T).astype(np.float16),
            "wqt": np.ascontiguousarray(weff["q"][hs:hs + 512, :].T).astype(np.float16),
            "wkt": np.ascontiguousarray(weff["k"][hs:hs + 512, :].T).astype(np.float16),
            "wvt": np.ascontiguousarray(weff["v"][hs:hs + 512, :].T).astype(np.float16),
            "wot": np.ascontiguousarray(weff["o"][:, hs:hs + 512].T).astype(np.float16),
            "bqs": np.ascontiguousarray(bq[hs:hs + 512]),
            "bks": np.ascontiguousarray(bk[hs:hs + 512]),
        })
    return maps


def _gather(inputs, results):
    wo, mo = np.asarray(inputs["wo"], np.float32), np.asarray(inputs["mo"], np.float32)
    bv, bo = np.asarray(inputs["bv"], np.float32), np.asarray(inputs["bo"], np.float32)
    out = np.zeros((B, T, C), np.float32)
    for b in range(B):
        out[b] = (results[2 * b]["out_part"].astype(np.float32)
                  + results[2 * b + 1]["out_part"].astype(np.float32))
    # host-side bias terms: v-bias flows through softmax (rows sum to 1) into
    # the o-projection; bo adds directly.
    out += (bv @ (wo * mo).T + bo)[None, None, :]
    return out


def kernel(**inputs):
    nc = _build_program()
    res = bass_utils.run_bass_kernel_spmd(nc, _in_maps(inputs),
                                          core_ids=list(range(NCORE)))
    return _gather(inputs, res.results)


def run_traced(**inputs):
    nc = _build_program()
    res = bass_utils.run_bass_kernel_spmd(nc, _in_maps(inputs),
                                          core_ids=list(range(NCORE)),
                                          trace=True)
    return _gather(inputs, res.results), res


# revision 37
# speedup vs baseline: 1.0063x; 1.0063x over previous
"""Trainium2 Bass kernel for nn_BrainAttention_69707319214147.

Model (reference.py): masked-weight QKV projections, per-row top-256-of-1024
sparsified attention scores, softmax over the scatter-into-zeros matrix
(zeros contribute exp(0)=1), AV, masked-weight output projection.

Sharding: 8 cores = 4 batches x 2 head-groups. Core i handles batch i//2 and
heads (i%2)*8 .. +8. Each core computes a partial output projection over its
512 y-channels; the host sums partner-core partials and adds bias terms.

Weight prep (w*mask fold, transposes, fp16 casts) happens on the host; the
device runs an all-fp16 pipeline (fp32 PSUM accumulation everywhere).

Per-head top-k softmax: tau approximates the 256th-largest raw score via a
Gaussian-quantile init (mean from the ACT copy accumulator, fixed global
sigma) plus 5 damped exact-count secant rounds - each a single fused fp16
compare+count tensor_scalar per row-block, 4x DVE mode. Then
u = (S >= tau)*S via a compare mask (DVE) and mask*S multiply (gpsimd),
dd = Exp(u/8) on ACT with the exact softmax denominator Z as the same
instruction's free accumulator (rejected entries give exp(0)=1 exactly),
dd scaled by 1/Z on DVE, transposed through the DMA xbar in one batched
[128,1024] call per row-block, and fed to AV / o-proj matmuls in fp16.
"""
import numpy as np
from collections import deque
from contextlib import ExitStack

import concourse.bass as bass
import concourse.mybir as mybir
import concourse.tile as tile
from concourse import bacc, bass_utils

F32 = mybir.dt.float32
F16 = mybir.dt.float16
AF = mybir.ActivationFunctionType
ALU = mybir.AluOpType

B, T, C, H = 4, 1024, 1024, 16
D = C // H            # 64
NCORE = 8
HPC = H // 2          # heads per core = 8
NT = T // 128         # 8 t-tiles
NCH = C // 128        # 8 contraction chunks
Z0 = 0.6744897501960817          # Phi^-1(0.75)
PHI0 = 0.3177765798295446        # phi(Z0)
SIGMA_GLOB = 4.0                 # global score-sigma estimate
SLOPE = SIGMA_GLOB / (T * PHI0)  # d(tau)/d(count)
TAU0_OFF = Z0 * SIGMA_GLOB
DAMPS = (1.0, 0.75, 0.5)  # damped secant update rounds


def _build_body(ctx, tc, io):
    nc = tc.nc
    P = 128

    # ---------------- persistent tiles ----------------
    pers = ctx.enter_context(tc.tile_pool(name="pers", bufs=1))

    # host-prepped fp16 effective weights + x
    weffq = pers.tile([P, NCH, 512], F16, tag="weffq")
    nc.sync.dma_start(weffq, io["wqt"].rearrange("(a p) f -> p a f", p=P))
    weffk = pers.tile([P, NCH, 512], F16, tag="weffk")
    nc.sync.dma_start(weffk, io["wkt"].rearrange("(a p) f -> p a f", p=P))
    xT16 = pers.tile([P, NCH, T], F16, tag="xT16")
    nc.sync.dma_start(xT16, io["xT"].rearrange("(a p) f -> p a f", p=P))

    bqc = pers.tile([P, 4], F32, tag="bqc")
    nc.sync.dma_start(bqc, io["bqs"].rearrange("(a p) -> p a", p=P))
    bkc = pers.tile([P, 4], F32, tag="bkc")
    nc.sync.dma_start(bkc, io["bks"].rearrange("(a p) -> p a", p=P))
    weffv = pers.tile([P, NCH, 512], F16, tag="weffv")
    nc.sync.dma_start(weffv, io["wvt"].rearrange("(a p) f -> p a f", p=P))
    weffo = pers.tile([P, 4, T], F16, tag="weffo")
    nc.sync.dma_start(weffo, io["wot"].rearrange("(a p) f -> p a f", p=P))

    qT = [pers.tile([P, T], F16, tag=f"qT{p}", name=f"qT{p}") for p in range(4)]
    kT = [pers.tile([P, T], F16, tag=f"kT{p}", name=f"kT{p}") for p in range(4)]
    vbf = [pers.tile([P, 512], F16, tag=f"v{ti}", name=f"v{ti}")
           for ti in range(NT)]
    yTp = [pers.tile([P, T], F16, tag=f"yTp{p}", name=f"yTp{p}")
           for p in range(4)]

    # ---------------- main loop: projections + attention ----------------
    with ExitStack() as c3:
        pps = c3.enter_context(tc.tile_pool(name="pps", bufs=2, space="PSUM"))
        pp2 = c3.enter_context(tc.tile_pool(name="pp2", bufs=1, space="PSUM"))
        avp = c3.enter_context(tc.tile_pool(name="avp", bufs=1, space="PSUM"))
        Spool = c3.enter_context(tc.tile_pool(name="Spool", bufs=17))
        cop = c3.enter_context(tc.tile_pool(name="cop", bufs=3))
        mpool = c3.enter_context(tc.tile_pool(name="mpool", bufs=2))
        upool = c3.enter_context(tc.tile_pool(name="upool", bufs=3))
        dpool = c3.enter_context(tc.tile_pool(name="dpool", bufs=4))
        dts = c3.enter_context(tc.tile_pool(name="dts", bufs=3))
        smp = c3.enter_context(tc.tile_pool(name="smp", bufs=4))
        zpool = c3.enter_context(tc.tile_pool(name="zpool", bufs=8))

        # PE backlog: proj/v/AV matmuls queue here and drain interleaved
        # between score tiles so long bursts never delay the score copies
        # that feed the ACT/DVE pipeline.
        backlog = deque()
        nfill = [0]
        ndrain = [0]

        def drain(n):
            while backlog and n > 0:
                backlog.popleft()()
                ndrain[0] += 1
                n -= 1

        def drain_to(mark):
            while ndrain[0] < mark and backlog:
                backlog.popleft()()
                ndrain[0] += 1

        def fill(thunk):
            backlog.append(thunk)
            nfill[0] += 1

        def fill_proj_quarter(p):
            for nm, wt, dst, bias in (("q", weffq, qT, bqc), ("k", weffk, kT, bkc)):
                ps = pp2.tile([P, T], F32, tag="ps2", name=f"ps2{nm}{p}")
                for cj in range(NCH):
                    for nh in range(2):
                        fill(lambda ps=ps, cj=cj, nh=nh, wt=wt: nc.tensor.matmul(
                            ps[:, nh * 512:(nh + 1) * 512],
                            lhsT=wt[:, cj, p * P:(p + 1) * P],
                            rhs=xT16[:, cj, nh * 512:(nh + 1) * 512],
                            start=(cj == 0), stop=(cj == NCH - 1),
                        ))
                fill(lambda ps=ps, dst=dst, bias=bias: nc.scalar.activation(
                    dst[p], ps, AF.Identity, bias=bias[:, p:p + 1], scale=1.0))
            return nfill[0]

        av_tiles = []

        def fill_av_half(yps, g, half):
            hh = 2 * g + half
            dts_h = av_tiles[hh]
            for nh in range(2):
                for j in range(NT):
                    fill(lambda yps=yps, half=half, hh=hh, nh=nh, j=j,
                         dts_h=dts_h: nc.tensor.matmul(
                        yps[64 * half:64 * half + 64,
                            nh * 512:(nh + 1) * 512],
                        lhsT=vbf[j][:, 64 * hh:64 * hh + 64],
                        rhs=dts_h[:, j, nh * 512:(nh + 1) * 512],
                        start=(j == 0), stop=(j == NT - 1),
                    ))

        def fill_av_pair(g):
            yps = avp.tile([P, T], F32, tag="yps", name=f"yps{g}")
            fill_av_half(yps, g, 0)
            fill_av_half(yps, g, 1)
            fill(lambda yps=yps, g=g: nc.scalar.copy(yTp[g], yps))

        def fill_v_pair(ti0):
            ps = pp2.tile([P, T], F32, tag="ps2", name=f"ps2v{ti0}")
            for half in range(2):
                ti = ti0 + half
                for cj in range(NCH):
                    fill(lambda ps=ps, half=half, ti=ti, cj=cj: nc.tensor.matmul(
                        ps[:, half * 512:(half + 1) * 512],
                        lhsT=xT16[:, cj, ti * P:(ti + 1) * P],
                        rhs=weffv[:, cj, :],
                        start=(cj == 0), stop=(cj == NCH - 1),
                    ))
                fill(lambda ps=ps, half=half, ti=ti: nc.scalar.copy(
                    vbf[ti], ps[:, half * 512:(half + 1) * 512]))

        def emit_scores(h):
            # copies for ti in dve_set are deferred to emit_dve_copies (run
            # on DVE after the previous head's counts) to balance ACT/DVE
            p, off = h // 2, 64 * (h % 2)
            if h == 0:
                dve_set = ()
            elif h % 2 == 0:
                dve_set = (6, 7)
            else:
                dve_set = (7,)
            Ssb = []
            pend = []
            for ti in range(NT):
                ps = pps.tile([P, T], F32, tag="ps")
                for nh in range(2):
                    nc.tensor.matmul(
                        ps[:, nh * 512:(nh + 1) * 512],
                        lhsT=qT[p][off:off + 64, ti * P:(ti + 1) * P],
                        rhs=kT[p][off:off + 64, nh * 512:(nh + 1) * 512],
                        start=True, stop=True,
                    )
                ssb = Spool.tile([P, T], F16, tag="ssb")
                if ti in dve_set:
                    pend.append((ssb, ps))
                else:
                    nc.scalar.copy(ssb, ps)
                Ssb.append(ssb)
                drain(7)
            return Ssb, pend

        def emit_dve_copies(pend):
            for ssb, ps in pend:
                nc.vector.tensor_copy(ssb, ps)

        # PE warm-up: dummy matmuls bridge the initial weight-DMA wait so the
        # tensor engine reaches full pstate before the first projection
        warm = Spool.tile([P, 512], F16, tag="warm", name="warm")
        nc.gpsimd.memset(warm, 0.0)
        wps = pp2.tile([P, 512], F32, tag="ps2", name="warmps")
        for _ in range(56):
            nc.tensor.matmul(wps, lhsT=warm[:, 0:128], rhs=warm,
                             start=True, stop=True)

        # prologue: head 0's production (quarter 0 runs eagerly; the PE is
        # idle during the weight DMAs anyway)
        fill_proj_quarter(0)
        drain(10 ** 9)
        fill_v_pair(0)
        fill_v_pair(2)
        Scur, pend0 = emit_scores(0)
        emit_dve_copies(pend0)

        proj_marks = {}
        for h in range(HPC):
            # fill the PE backlog for upcoming blocks: quarter p two blocks
            # ahead of its first consumer, AV pair g one block after its
            # second head completes
            if h == 0:
                fill_v_pair(4)
                fill_v_pair(6)
                proj_marks[1] = fill_proj_quarter(1)
            elif h in (2, 4):
                proj_marks[h // 2 + 1] = fill_proj_quarter(h // 2 + 1)
            if h >= 2 and h % 2 == 0:
                fill_av_pair((h - 2) // 2)
            if h == HPC - 1:
                # last pair: head-6 half can run while head 7's tail drains
                av_last = avp.tile([P, T], F32, tag="yps", name="yps3")
                fill_av_half(av_last, HPC // 2 - 1, 0)

            pend_next = []
            if h + 1 < HPC:
                if (h + 1) % 2 == 0:
                    drain_to(proj_marks[(h + 1) // 2])
                Snext, pend_next = emit_scores(h + 1)
            Ssb = Scur

            # --- damped exact-count secant rounds (fixed global init) ---
            tau = None
            for r, damp in enumerate(DAMPS):
                cnt = smp.tile([P, NT], F32, tag="cnt")
                for ti in range(NT):
                    jk = cop.tile([P, T], F16, tag="jk")
                    nc.vector.tensor_scalar(
                        jk, Ssb[ti],
                        TAU0_OFF if r == 0 else tau[:, ti:ti + 1], None,
                        op0=ALU.is_ge, op1=ALU.add,
                        accum_out=cnt[:, ti:ti + 1])
                dl = smp.tile([P, NT], F32, tag="dl")
                nc.vector.tensor_scalar(dl, cnt, -256.0, damp * SLOPE,
                                        op0=ALU.add, op1=ALU.mult)
                tau2 = smp.tile([P, NT], F32, tag="tau")
                if r == 0:
                    nc.vector.tensor_scalar_add(tau2, dl, TAU0_OFF)
                else:
                    nc.vector.tensor_add(tau2, tau, dl)
                tau = tau2

            # --- select, exp (with exact Z), normalize, transpose ---
            DTs = dts.tile([P, NT, T], F16, tag="dts")
            zacc = smp.tile([P, NT], F32, tag="zacc")
            for ti in range(NT):
                mk = mpool.tile([P, T], F16, tag="mk")
                nc.vector.tensor_scalar(mk, Ssb[ti], tau[:, ti:ti + 1], None,
                                        op0=ALU.is_ge)
                u = upool.tile([P, T], F16, tag="u")
                nc.vector.tensor_tensor(out=u, in0=mk, in1=Ssb[ti],
                                        op=ALU.mult)
                dd = dpool.tile([P, T], F16, tag="dd")
                nc.scalar.activation(dd, u, AF.Exp, scale=0.125,
                                     accum_out=zacc[:, ti:ti + 1])
                zinv = zpool.tile([P, 1], F32, tag="zinv")
                nc.vector.reciprocal(zinv, zacc[:, ti:ti + 1])
                if ti % 2 == 0 or h == HPC - 1:
                    nc.gpsimd.tensor_scalar_mul(dd, dd, zinv[:, 0:1])
                else:
                    nc.vector.tensor_scalar_mul(dd, dd, zinv[:, 0:1])
                nc.sync.dma_start_transpose(
                    DTs[:, :, ti * P:(ti + 1) * P], dd)

            # deferred DVE copies for head h+1 (end of block: psum ready)
            emit_dve_copies(pend_next)

            av_tiles.append(DTs)
            if h == HPC - 1:
                fill_av_half(av_last, HPC // 2 - 1, 1)
                fill(lambda: nc.scalar.copy(yTp[HPC // 2 - 1], av_last))
                drain(10 ** 9)
            if h + 1 < HPC:
                Scur = Snext

    # ---------------- output projection ----------------
    with ExitStack() as c4:
        ops4 = c4.enter_context(tc.tile_pool(name="ops4", bufs=2, space="PSUM"))
        ost4 = c4.enter_context(tc.tile_pool(name="ost4", bufs=2))
        for ti in range(NT):
            ps = ops4.tile([P, T], F32, tag="ops")
            for cj in range(4):
                for nh in range(2):
                    nc.tensor.matmul(
                        ps[:, nh * 512:(nh + 1) * 512],
                        lhsT=yTp[cj][:, ti * P:(ti + 1) * P],
                        rhs=weffo[:, cj, nh * 512:(nh + 1) * 512],
                        start=(cj == 0), stop=(cj == 3),
                    )
            ost = ost4.tile([P, T], F16, tag="ost")
            nc.scalar.copy(ost, ps)
            nc.sync.dma_start(io["out_part"][ti * P:(ti + 1) * P, :], ost)


_PROG_CACHE = {}


def _build_program():
    if "nc" in _PROG_CACHE:
        return _PROG_CACHE["nc"]
    nc = bacc.Bacc("TRN2", target_bir_lowering=False, debug=False)
    io = {}
    io["xT"] = nc.dram_tensor("xT", [C, T], F16, kind="ExternalInput").ap()
    for nm in ("q", "k", "v"):
        io[f"w{nm}t"] = nc.dram_tensor(f"w{nm}t", [C, 512], F16,
                                       kind="ExternalInput").ap()
    io["wot"] = nc.dram_tensor("wot", [512, C], F16, kind="ExternalInput").ap()
    io["bqs"] = nc.dram_tensor("bqs", [512], F32, kind="ExternalInput").ap()
    io["bks"] = nc.dram_tensor("bks", [512], F32, kind="ExternalInput").ap()
    io["out_part"] = nc.dram_tensor("out_part", [T, C], F16,
                                    kind="ExternalOutput").ap()
    with tile.TileContext(nc) as tc:
        with ExitStack() as ctx:
            _build_body(ctx, tc, io)
    nc.compile()
    _PROG_CACHE["nc"] = nc
    return nc


def _in_maps(inputs):
    f32 = np.float32
    x = np.asarray(inputs["x"], f32)
    weff = {}
    for nm in ("q", "k", "v", "o"):
        weff[nm] = (np.asarray(inputs["w" + nm], f32)
                    * np.asarray(inputs["m" + nm], f32))
    bq, bk = np.asarray(inputs["bq"], f32), np.asarray(inputs["bk"], f32)
    maps = []
    for core in range(NCORE):
        b, g = core // 2, core % 2
        hs = g * 512
        maps.append({
            "xT": np.ascontiguousarray(x[b].T).astype(np.float16),
            "wqt": np.ascontiguousarray(weff["q"][hs:hs + 512, :].T).astype(np.float16),
            "wkt": np.ascontiguousarray(weff["k"][hs:hs + 512, :].T).astype(np.float16),
            "wvt": np.ascontiguousarray(weff["v"][hs:hs + 512, :].T).astype(np.float16),
            "wot": np.ascontiguousarray(weff["o"][:, hs:hs + 512].T).astype(np.float16),
            "bqs": np.ascontiguousarray(bq[hs:hs + 512]),
            "bks": np.ascontiguousarray(bk[hs:hs + 512]),
        })
    return maps


def _gather(inputs, results):
    wo, mo = np.asarray(inputs["wo"], np.float32), np.asarray(inputs["mo"], np.float32)
    bv, bo = np.asarray(inputs["bv"], np.float32), np.asarray(inputs["bo"], np.float32)
    out = np.zeros((B, T, C), np.float32)
    for b in range(B):
        out[b] = (results[2 * b]["out_part"].astype(np.float32)
                  + results[2 * b + 1]["out_part"].astype(np.float32))
    # host-side bias terms: v-bias flows through softmax (rows sum to 1) into
    # the o-projection; bo adds directly.
    out += (bv @ (wo * mo).T + bo)[None, None, :]
    return out


def kernel(**inputs):
    nc = _build_program()
    res = bass_utils.run_bass_kernel_spmd(nc, _in_maps(inputs),
                                          core_ids=list(range(NCORE)))
    return _gather(inputs, res.results)


def run_traced(**inputs):
    nc = _build_program()
    res = bass_utils.run_bass_kernel_spmd(nc, _in_maps(inputs),
                                          core_ids=list(range(NCORE)),
                                          trace=True)
    return _gather(inputs, res.results), res


# revision 39
# speedup vs baseline: 1.0165x; 1.0102x over previous
"""Trainium2 Bass kernel for nn_BrainAttention_69707319214147.

Model (reference.py): masked-weight QKV projections, per-row top-256-of-1024
sparsified attention scores, softmax over the scatter-into-zeros matrix
(zeros contribute exp(0)=1), AV, masked-weight output projection.

Sharding: 8 cores = 4 batches x 2 head-groups. Core i handles batch i//2 and
heads (i%2)*8 .. +8. Each core computes a partial output projection over its
512 y-channels; the host sums partner-core partials and adds bias terms.

Weight prep (w*mask fold, transposes, fp16 casts) happens on the host; the
device runs an all-fp16 pipeline (fp32 PSUM accumulation everywhere).

Per-head top-k softmax: tau approximates the 256th-largest raw score via a
Gaussian-quantile init (mean from the ACT copy accumulator, fixed global
sigma) plus 5 damped exact-count secant rounds - each a single fused fp16
compare+count tensor_scalar per row-block, 4x DVE mode. Then
u = (S >= tau)*S via a compare mask (DVE) and mask*S multiply (gpsimd),
dd = Exp(u/8) on ACT with the exact softmax denominator Z as the same
instruction's free accumulator (rejected entries give exp(0)=1 exactly),
dd scaled by 1/Z on DVE, transposed through the DMA xbar in one batched
[128,1024] call per row-block, and fed to AV / o-proj matmuls in fp16.
"""
import numpy as np
from collections import deque
from contextlib import ExitStack

import concourse.bass as bass
import concourse.mybir as mybir
import concourse.tile as tile
from concourse import bacc, bass_utils

F32 = mybir.dt.float32
F16 = mybir.dt.float16
AF = mybir.ActivationFunctionType
ALU = mybir.AluOpType

B, T, C, H = 4, 1024, 1024, 16
D = C // H            # 64
NCORE = 8
HPC = H // 2          # heads per core = 8
NT = T // 128         # 8 t-tiles
NCH = C // 128        # 8 contraction chunks
Z0 = 0.6744897501960817          # Phi^-1(0.75)
PHI0 = 0.3177765798295446        # phi(Z0)
SIGMA_GLOB = 4.0                 # global score-sigma estimate
SLOPE = SIGMA_GLOB / (T * PHI0)  # d(tau)/d(count)
TAU0_OFF = Z0 * SIGMA_GLOB
DAMPS = (1.0, 0.75, 0.5)  # damped secant update rounds


def _build_body(ctx, tc, io):
    nc = tc.nc
    P = 128

    # ---------------- persistent tiles ----------------
    pers = ctx.enter_context(tc.tile_pool(name="pers", bufs=1))

    # host-prepped fp16 effective weights + x
    weffq = pers.tile([P, NCH, 512], F16, tag="weffq")
    nc.sync.dma_start(weffq, io["wqt"].rearrange("(a p) f -> p a f", p=P))
    weffk = pers.tile([P, NCH, 512], F16, tag="weffk")
    nc.sync.dma_start(weffk, io["wkt"].rearrange("(a p) f -> p a f", p=P))
    xT16 = pers.tile([P, NCH, T], F16, tag="xT16")
    nc.sync.dma_start(xT16, io["xT"].rearrange("(a p) f -> p a f", p=P))

    bqc = pers.tile([P, 4], F32, tag="bqc")
    nc.sync.dma_start(bqc, io["bqs"].rearrange("(a p) -> p a", p=P))
    bkc = pers.tile([P, 4], F32, tag="bkc")
    nc.sync.dma_start(bkc, io["bks"].rearrange("(a p) -> p a", p=P))
    weffv = pers.tile([P, NCH, 512], F16, tag="weffv")
    nc.sync.dma_start(weffv, io["wvt"].rearrange("(a p) f -> p a f", p=P))
    weffo = pers.tile([P, 4, T], F16, tag="weffo")
    nc.sync.dma_start(weffo, io["wot"].rearrange("(a p) f -> p a f", p=P))

    qT = [pers.tile([P, T], F16, tag=f"qT{p}", name=f"qT{p}") for p in range(4)]
    kT = [pers.tile([P, T], F16, tag=f"kT{p}", name=f"kT{p}") for p in range(4)]
    vbf = [pers.tile([P, 512], F16, tag=f"v{ti}", name=f"v{ti}")
           for ti in range(NT)]
    yTp = [pers.tile([P, T], F16, tag=f"yTp{p}", name=f"yTp{p}")
           for p in range(4)]

    # ---------------- main loop: projections + attention ----------------
    with ExitStack() as c3:
        pps = c3.enter_context(tc.tile_pool(name="pps", bufs=2, space="PSUM"))
        pp2 = c3.enter_context(tc.tile_pool(name="pp2", bufs=1, space="PSUM"))
        avp = c3.enter_context(tc.tile_pool(name="avp", bufs=1, space="PSUM"))
        Spool = c3.enter_context(tc.tile_pool(name="Spool", bufs=17))
        cop = c3.enter_context(tc.tile_pool(name="cop", bufs=3))
        mpool = c3.enter_context(tc.tile_pool(name="mpool", bufs=2))
        upool = c3.enter_context(tc.tile_pool(name="upool", bufs=3))
        dpool = c3.enter_context(tc.tile_pool(name="dpool", bufs=4))
        dts = c3.enter_context(tc.tile_pool(name="dts", bufs=3))
        smp = c3.enter_context(tc.tile_pool(name="smp", bufs=4))
        zpool = c3.enter_context(tc.tile_pool(name="zpool", bufs=8))

        # PE backlog: proj/v/AV matmuls queue here and drain interleaved
        # between score tiles so long bursts never delay the score copies
        # that feed the ACT/DVE pipeline.
        backlog = deque()
        nfill = [0]
        ndrain = [0]

        def drain(n):
            while backlog and n > 0:
                backlog.popleft()()
                ndrain[0] += 1
                n -= 1

        def drain_to(mark):
            while ndrain[0] < mark and backlog:
                backlog.popleft()()
                ndrain[0] += 1

        def fill(thunk):
            backlog.append(thunk)
            nfill[0] += 1

        def fill_proj_quarter(p):
            for nm, wt, dst, bias in (("q", weffq, qT, bqc), ("k", weffk, kT, bkc)):
                ps = pp2.tile([P, T], F32, tag="ps2", name=f"ps2{nm}{p}")
                for cj in range(NCH):
                    for nh in range(2):
                        fill(lambda ps=ps, cj=cj, nh=nh, wt=wt: nc.tensor.matmul(
                            ps[:, nh * 512:(nh + 1) * 512],
                            lhsT=wt[:, cj, p * P:(p + 1) * P],
                            rhs=xT16[:, cj, nh * 512:(nh + 1) * 512],
                            start=(cj == 0), stop=(cj == NCH - 1),
                        ))
                fill(lambda ps=ps, dst=dst, bias=bias: nc.scalar.activation(
                    dst[p], ps, AF.Identity, bias=bias[:, p:p + 1], scale=1.0))
            return nfill[0]

        av_tiles = []

        def fill_av_half(yps, g, half):
            hh = 2 * g + half
            dts_h = av_tiles[hh]
            for nh in range(2):
                for j in range(NT):
                    fill(lambda yps=yps, half=half, hh=hh, nh=nh, j=j,
                         dts_h=dts_h: nc.tensor.matmul(
                        yps[64 * half:64 * half + 64,
                            nh * 512:(nh + 1) * 512],
                        lhsT=vbf[j][:, 64 * hh:64 * hh + 64],
                        rhs=dts_h[:, j, nh * 512:(nh + 1) * 512],
                        start=(j == 0), stop=(j == NT - 1),
                    ))

        def fill_av_pair(g):
            yps = avp.tile([P, T], F32, tag="yps", name=f"yps{g}")
            fill_av_half(yps, g, 0)
            fill_av_half(yps, g, 1)
            fill(lambda yps=yps, g=g: nc.scalar.copy(yTp[g], yps))

        def fill_v_pair(ti0):
            ps = pp2.tile([P, T], F32, tag="ps2", name=f"ps2v{ti0}")
            for half in range(2):
                ti = ti0 + half
                for cj in range(NCH):
                    fill(lambda ps=ps, half=half, ti=ti, cj=cj: nc.tensor.matmul(
                        ps[:, half * 512:(half + 1) * 512],
                        lhsT=xT16[:, cj, ti * P:(ti + 1) * P],
                        rhs=weffv[:, cj, :],
                        start=(cj == 0), stop=(cj == NCH - 1),
                    ))
                fill(lambda ps=ps, half=half, ti=ti: nc.scalar.copy(
                    vbf[ti], ps[:, half * 512:(half + 1) * 512]))

        def emit_scores(h):
            # copies for ti in dve_set are deferred to emit_dve_copies (run
            # on DVE after the previous head's counts) to balance ACT/DVE
            p, off = h // 2, 64 * (h % 2)
            if h in (0, HPC - 1):
                dve_set = ()
            elif h % 2 == 0:
                dve_set = (6, 7)
            else:
                dve_set = (7,)
            Ssb = []
            pend = []
            for ti in range(NT):
                ps = pps.tile([P, T], F32, tag="ps")
                for nh in range(2):
                    nc.tensor.matmul(
                        ps[:, nh * 512:(nh + 1) * 512],
                        lhsT=qT[p][off:off + 64, ti * P:(ti + 1) * P],
                        rhs=kT[p][off:off + 64, nh * 512:(nh + 1) * 512],
                        start=True, stop=True,
                    )
                ssb = Spool.tile([P, T], F16, tag="ssb")
                if ti in dve_set:
                    pend.append((ssb, ps))
                else:
                    nc.scalar.copy(ssb, ps)
                Ssb.append(ssb)
                drain(7)
            return Ssb, pend

        def emit_dve_copies(pend):
            for ssb, ps in pend:
                nc.vector.tensor_copy(ssb, ps)

        # PE warm-up: dummy matmuls bridge the initial weight-DMA wait so the
        # tensor engine reaches full pstate before the first projection
        warm = Spool.tile([P, 512], F16, tag="warm", name="warm")
        nc.gpsimd.memset(warm, 0.0)
        wps = pp2.tile([P, 512], F32, tag="ps2", name="warmps")
        for _ in range(56):
            nc.tensor.matmul(wps, lhsT=warm[:, 0:128], rhs=warm,
                             start=True, stop=True)

        # prologue: head 0's production (quarter 0 runs eagerly; the PE is
        # idle during the weight DMAs anyway)
        fill_proj_quarter(0)
        drain(10 ** 9)
        fill_v_pair(0)
        fill_v_pair(2)
        Scur, pend0 = emit_scores(0)
        emit_dve_copies(pend0)

        proj_marks = {}
        for h in range(HPC):
            # fill the PE backlog for upcoming blocks: quarter p two blocks
            # ahead of its first consumer, AV pair g one block after its
            # second head completes
            if h == 0:
                fill_v_pair(4)
                fill_v_pair(6)
                proj_marks[1] = fill_proj_quarter(1)
            elif h in (2, 4):
                proj_marks[h // 2 + 1] = fill_proj_quarter(h // 2 + 1)
            if h >= 2 and h % 2 == 0:
                fill_av_pair((h - 2) // 2)
            if h == HPC - 1:
                # last pair: head-6 half can run while head 7's tail drains
                av_last = avp.tile([P, T], F32, tag="yps", name="yps3")
                fill_av_half(av_last, HPC // 2 - 1, 0)

            pend_next = []
            if h + 1 < HPC:
                if (h + 1) % 2 == 0:
                    drain_to(proj_marks[(h + 1) // 2])
                Snext, pend_next = emit_scores(h + 1)
            Ssb = Scur

            # --- damped exact-count secant rounds (fixed global init) ---
            tau = None
            for r, damp in enumerate(DAMPS):
                cnt = smp.tile([P, NT], F32, tag="cnt")
                for ti in range(NT):
                    jk = cop.tile([P, T], F16, tag="jk")
                    nc.vector.tensor_scalar(
                        jk, Ssb[ti],
                        TAU0_OFF if r == 0 else tau[:, ti:ti + 1], None,
                        op0=ALU.is_ge, op1=ALU.add,
                        accum_out=cnt[:, ti:ti + 1])
                dl = smp.tile([P, NT], F32, tag="dl")
                nc.vector.tensor_scalar(dl, cnt, -256.0, damp * SLOPE,
                                        op0=ALU.add, op1=ALU.mult)
                tau2 = smp.tile([P, NT], F32, tag="tau")
                if r == 0:
                    nc.vector.tensor_scalar_add(tau2, dl, TAU0_OFF)
                else:
                    nc.vector.tensor_add(tau2, tau, dl)
                tau = tau2

            # --- select, exp (with exact Z), normalize, transpose ---
            DTs = dts.tile([P, NT, T], F16, tag="dts")
            zacc = smp.tile([P, NT], F32, tag="zacc")
            for ti in range(NT):
                mk = mpool.tile([P, T], F16, tag="mk")
                nc.vector.tensor_scalar(mk, Ssb[ti], tau[:, ti:ti + 1], None,
                                        op0=ALU.is_ge)
                u = upool.tile([P, T], F16, tag="u")
                nc.vector.tensor_tensor(out=u, in0=mk, in1=Ssb[ti],
                                        op=ALU.mult)
                dd = dpool.tile([P, T], F16, tag="dd")
                nc.scalar.activation(dd, u, AF.Exp, scale=0.125,
                                     accum_out=zacc[:, ti:ti + 1])
                zinv = zpool.tile([P, 1], F32, tag="zinv")
                nc.vector.reciprocal(zinv, zacc[:, ti:ti + 1])
                if ti % 2 == 0 and h != HPC - 1:
                    nc.gpsimd.tensor_scalar_mul(dd, dd, zinv[:, 0:1])
                else:
                    nc.vector.tensor_scalar_mul(dd, dd, zinv[:, 0:1])
                nc.sync.dma_start_transpose(
                    DTs[:, :, ti * P:(ti + 1) * P], dd)

            # deferred DVE copies for head h+1 (end of block: psum ready)
            emit_dve_copies(pend_next)

            av_tiles.append(DTs)
            if h == HPC - 1:
                fill_av_half(av_last, HPC // 2 - 1, 1)
                fill(lambda: nc.scalar.copy(yTp[HPC // 2 - 1], av_last))
                drain(10 ** 9)
            if h + 1 < HPC:
                Scur = Snext

    # ---------------- output projection ----------------
    with ExitStack() as c4:
        ops4 = c4.enter_context(tc.tile_pool(name="ops4", bufs=2, space="PSUM"))
        ost4 = c4.enter_context(tc.tile_pool(name="ost4", bufs=2))
        for ti in range(NT):
            ps = ops4.tile([P, T], F32, tag="ops")
            for cj in range(4):
                for nh in range(2):
                    nc.tensor.matmul(
                        ps[:, nh * 512:(nh + 1) * 512],
                        lhsT=yTp[cj][:, ti * P:(ti + 1) * P],
                        rhs=weffo[:, cj, nh * 512:(nh + 1) * 512],
                        start=(cj == 0), stop=(cj == 3),
                    )
            ost = ost4.tile([P, T], F16, tag="ost")
            nc.scalar.copy(ost, ps)
            nc.sync.dma_start(io["out_part"][ti * P:(ti + 1) * P, :], ost)


_PROG_CACHE = {}


def _build_program():
    if "nc" in _PROG_CACHE:
        return _PROG_CACHE["nc"]
    nc = bacc.Bacc("TRN2", target_bir_lowering=False, debug=False)
    io = {}
    io["xT"] = nc.dram_tensor("xT", [C, T], F16, kind="ExternalInput").ap()
    for nm in ("q", "k", "v"):
        io[f"w{nm}t"] = nc.dram_tensor(f"w{nm}t", [C, 512], F16,
                                       kind="ExternalInput").ap()
    io["wot"] = nc.dram_tensor("wot", [512, C], F16, kind="ExternalInput").ap()
    io["bqs"] = nc.dram_tensor("bqs", [512], F32, kind="ExternalInput").ap()
    io["bks"] = nc.dram_tensor("bks", [512], F32, kind="ExternalInput").ap()
    io["out_part"] = nc.dram_tensor("out_part", [T, C], F16,
                                    kind="ExternalOutput").ap()
    with tile.TileContext(nc) as tc:
        with ExitStack() as ctx:
            _build_body(ctx, tc, io)
    nc.compile()
    _PROG_CACHE["nc"] = nc
    return nc


def _in_maps(inputs):
    f32 = np.float32
    x = np.asarray(inputs["x"], f32)
    weff = {}
    for nm in ("q", "k", "v", "o"):
        weff[nm] = (np.asarray(inputs["w" + nm], f32)
                    * np.asarray(inputs["m" + nm], f32))
    bq, bk = np.asarray(inputs["bq"], f32), np.asarray(inputs["bk"], f32)
    maps = []
    for core in range(NCORE):
        b, g = core // 2, core % 2
        hs = g * 512
        maps.append({
            "xT": np.ascontiguousarray(x[b].T).astype(np.float16),
            "wqt": np.ascontiguousarray(weff["q"][hs:hs + 512, :].T).astype(np.float16),
            "wkt": np.ascontiguousarray(weff["k"][hs:hs + 512, :].T).astype(np.float16),
            "wvt": np.ascontiguousarray(weff["v"][hs:hs + 512, :].T).astype(np.float16),
            "wot": np.ascontiguousarray(weff["o"][:, hs:hs + 512].T).astype(np.float16),
            "bqs": np.ascontiguousarray(bq[hs:hs + 512]),
            "bks": np.ascontiguousarray(bk[hs:hs + 512]),
        })
    return maps


def _gather(inputs, results):
    wo, mo = np.asarray(inputs["wo"], np.float32), np.asarray(inputs["mo"], np.float32)
    bv, bo = np.asarray(inputs["bv"], np.float32), np.asarray(inputs["bo"], np.float32)
    out = np.zeros((B, T, C), np.float32)
    for b in range(B):
        out[b] = (results[2 * b]["out_part"].astype(np.float32)
                  + results[2 * b + 1]["out_part"].astype(np.float32))
    # host-side bias terms: v-bias flows through softmax (rows sum to 1) into
    # the o-projection; bo adds directly.
    out += (bv @ (wo * mo).T + bo)[None, None, :]
    return out


def kernel(**inputs):
    nc = _build_program()
    res = bass_utils.run_bass_kernel_spmd(nc, _in_maps(inputs),
                                          core_ids=list(range(NCORE)))
    return _gather(inputs, res.results)


def run_traced(**inputs):
    nc = _build_program()
    res = bass_utils.run_bass_kernel_spmd(nc, _in_maps(inputs),
                                          core_ids=list(range(NCORE)),
                                          trace=True)
    return _gather(inputs, res.results), res
